# revision 7
# baseline (speedup 1.0000x reference)
"""Multi-head attention (batch=2, seq=2048, d_model=2048, 16 heads, causal)
on 8 Trainium2 NeuronCores.

Sharding (Megatron-style tensor parallel + data parallel):
  core c -> batch b = c // 4, feature block j = c % 4 (4 heads = 512 features).
  Each core computes Q/K/V projections for its 512 feature columns
  (w_q/w_k/w_v column-sliced), attention for its 4 heads, and a partial
  output projection (w_o row-sliced).  The 4 partial outputs per batch
  element are summed on the host (the Megatron row-parallel AllReduce).

Device math (per core), all matmuls in bf16 with fp32 PSUM accumulation:
  xT  = x[b].T                          [2048 dm, 2048 s]   (host-prepped)
  Q^T = wq_c.T @ ... -> lhsT=wq chunks  [512 f, 2048 s]
  K^T                                    [512 f, 2048 s]
  V   = x @ wv_c                         [2048 s, 512 f]    (+ ones column)
  per head h, per key block kc:  S^T[k, q] = K^T_h[:,kc].T @ Q^T_h
  T = exp(S^T / sqrt(128))  (unnormalized softmax; scores are O(5) so no
      max-subtraction is needed in fp32), causal-masked
  per query block qb: O[q, d|sum] = sum_kc T_kc[:, qb].T @ [V_kc | 1]
  O /= sum  -> transpose via PE -> O^T [512 f, 2048 s]
  out partial = O^T.T @ wo_c             [2048 s, 2048 dmo]  fp32
"""

import math
import threading
from contextlib import ExitStack

import ml_dtypes
import numpy as np

import concourse.bass as bass
import concourse.mybir as mybir
import concourse.tile as tile
from concourse import bacc
from concourse.masks import make_identity

BF16 = mybir.dt.float16
F32 = mybir.dt.float32
NPBF16 = np.float16

SEQ = 2048
DM = 2048
HEADS_PER_CORE = 4
F = 512  # features per core
P = 128
NKC = SEQ // P  # 16 key blocks
NR = DM // P  # 16 contraction chunks
SCALE = 1.0 / math.sqrt(128.0)

# compact T-buffer offsets: block kc covers q in [kc*128, 2048)
T_WIDTHS = [SEQ - kc * P for kc in range(NKC)]
T_OFFS = list(np.cumsum([0] + T_WIDTHS[:-1]))
T_TOTAL = int(np.sum(T_WIDTHS))  # 17408


def build_nc(iters: int = 1, rep_a: int = 1, rep_b: int = 1, rep_c: int = 1) -> bacc.Bacc:
    nc = bacc.Bacc("TRN2", num_devices=8)

    xt_h = nc.dram_tensor("xt", [DM, SEQ], BF16, kind="ExternalInput")
    wq_h = nc.dram_tensor("wq", [DM, F], BF16, kind="ExternalInput")
    wk_h = nc.dram_tensor("wk", [DM, F], BF16, kind="ExternalInput")
    wv_h = nc.dram_tensor("wv", [DM, F], BF16, kind="ExternalInput")
    wo_h = nc.dram_tensor("wo", [F, DM], BF16, kind="ExternalInput")
    tri_h = nc.dram_tensor("tri", [P, P], BF16, kind="ExternalInput")
    out_h = nc.dram_tensor("out", [SEQ, DM], F32, kind="ExternalOutput")

    xt = xt_h.ap()
    wo_r = wo_h.ap().rearrange("(c p) n -> p c n", p=P)  # [128, 4, 2048]
    out_ap = out_h.ap()

    with tile.TileContext(nc) as tc, ExitStack() as octx:
        consts = octx.enter_context(tc.tile_pool(name="consts", bufs=1))
        ident = consts.tile([P, P], BF16)
        make_identity(nc, ident)
        tri_sb = consts.tile([P, P], BF16)
        nc.sync.dma_start(out=tri_sb, in_=tri_h.ap())

        for _ in range(iters):
            with ExitStack() as ictx:
                persist = ictx.enter_context(tc.tile_pool(name="persist", bufs=1))
                qt_sb = persist.tile([P, HEADS_PER_CORE, SEQ], BF16)
                kt_sb = persist.tile([P, HEADS_PER_CORE, SEQ], BF16)
                v_sb = persist.tile([P, NKC, HEADS_PER_CORE, P + 1], BF16)
                ot_sb = persist.tile([P, HEADS_PER_CORE, SEQ], BF16)
                wo_sb = persist.tile([P, HEADS_PER_CORE, DM], BF16)

                nc.sync.dma_start(out=wo_sb, in_=wo_r)
                # ones column for the fused softmax-denominator trick
                nc.vector.memset(v_sb[:, :, :, P : P + 1], 1.0)

                # ---------------- Phase A: projections ----------------
                for _ra in range(rep_a):
                  with ExitStack() as actx:
                    pa = actx.enter_context(tc.tile_pool(name="pa", bufs=1))
                    wpool = actx.enter_context(tc.tile_pool(name="wpool", bufs=2))
                    psq = actx.enter_context(
                        tc.tile_pool(name="psq", bufs=3, space="PSUM")
                    )
                    psv = actx.enter_context(
                        tc.tile_pool(name="psv", bufs=2, space="PSUM")
                    )

                    xt_sb = pa.tile([P, NR, SEQ], BF16)
                    for r in range(NR):
                        nc.sync.dma_start(
                            out=xt_sb[:, r, :], in_=xt[r * P : (r + 1) * P, :]
                        )

                    # Q^T and K^T: psum[f_rel, s] = sum_r w[r,f].T @ xT[r, s]
                    for w_h, dst in ((wq_h, qt_sb), (wk_h, kt_sb)):
                        w_r = w_h.ap().rearrange("(r p) f -> p r f", p=P)
                        w_t = wpool.tile([P, NR, F], BF16, tag="w")
                        for rr in range(4):
                            nc.sync.dma_start(
                                out=w_t[:, rr * 4 : (rr + 1) * 4, :],
                                in_=w_r[:, rr * 4 : (rr + 1) * 4, :],
                            )
                        for f in range(HEADS_PER_CORE):
                            pq0 = psq.tile([P, 1024], F32, tag="psq")
                            pq1 = psq.tile([P, 1024], F32, tag="psq")
                            for r in range(NR):
                                lhsT = w_t[:, r, f * P : (f + 1) * P]
                                for half, pq in ((0, pq0), (1, pq1)):
                                    for sn in range(2):
                                        s0 = half * 1024 + sn * 512
                                        nc.tensor.matmul(
                                            pq[:, sn * 512 : (sn + 1) * 512],
                                            lhsT,
                                            xt_sb[:, r, s0 : s0 + 512],
                                            start=(r == 0),
                                            stop=(r == NR - 1),
                                        )
                            nc.vector.tensor_copy(dst[:, f, 0:1024], pq0)
                            nc.scalar.copy(dst[:, f, 1024:2048], pq1)

                    # V: psum[s_rel, f] = sum_r xT[r, s].T @ wv[r, f]
                    wv_r = wv_h.ap().rearrange("(r p) f -> p r f", p=P)
                    wv_t = wpool.tile([P, NR, F], BF16, tag="w")
                    for rr in range(4):
                        nc.sync.dma_start(
                            out=wv_t[:, rr * 4 : (rr + 1) * 4, :],
                            in_=wv_r[:, rr * 4 : (rr + 1) * 4, :],
                        )
                    for sm in range(NKC):
                        pv = psv.tile([P, F], F32, tag="psv")
                        for r in range(NR):
                            nc.tensor.matmul(
                                pv,
                                xt_sb[:, r, sm * P : (sm + 1) * P],
                                wv_t[:, r, :],
                                start=(r == 0),
                                stop=(r == NR - 1),
                            )
                        eng = nc.vector if sm % 2 == 0 else nc.scalar
                        if eng is nc.vector:
                            eng.tensor_copy(
                                v_sb[:, sm, :, 0:P],
                                pv.rearrange("p (h d) -> p h d", h=HEADS_PER_CORE),
                            )
                        else:
                            eng.copy(
                                v_sb[:, sm, :, 0:P],
                                pv.rearrange("p (h d) -> p h d", h=HEADS_PER_CORE),
                            )

                # ---------------- Phase B: attention ----------------
                for _rb in range(rep_b):
                  with ExitStack() as bctx:
                    pb = bctx.enter_context(tc.tile_pool(name="pb", bufs=2))
                    pbo = bctx.enter_context(tc.tile_pool(name="pbo", bufs=3))
                    pss = bctx.enter_context(
                        tc.tile_pool(name="pss", bufs=2, space="PSUM")
                    )
                    pso = bctx.enter_context(
                        tc.tile_pool(name="pso", bufs=2, space="PSUM")
                    )
                    pst = bctx.enter_context(
                        tc.tile_pool(name="pst", bufs=2, space="PSUM")
                    )

                    for h in range(HEADS_PER_CORE):
                        t_h = pb.tile([P, T_TOTAL], BF16, tag="T")
                        # pass 1: T = causal_mask(exp(S^T / sqrt(d)))
                        for kc in range(NKC):
                            w = T_WIDTHS[kc]
                            off = T_OFFS[kc]
                            q0 = kc * P
                            lhsT = kt_sb[:, h, kc * P : (kc + 1) * P]
                            for c in range((w + 1023) // 1024):
                                wc = min(1024, w - c * 1024)
                                ps = pss.tile([P, 1024], F32, tag="pss")
                                for n in range((wc + 511) // 512):
                                    nw = min(512, wc - n * 512)
                                    nc.tensor.matmul(
                                        ps[:, n * 512 : n * 512 + nw],
                                        lhsT,
                                        qt_sb[
                                            :,
                                            h,
                                            q0 + c * 1024 + n * 512 : q0
                                            + c * 1024
                                            + n * 512
                                            + nw,
                                        ],
                                        start=True,
                                        stop=True,
                                    )
                                nc.scalar.activation(
                                    t_h[:, off + c * 1024 : off + c * 1024 + wc],
                                    ps[:, 0:wc],
                                    mybir.ActivationFunctionType.Exp,
                                    scale=SCALE,
                                )
                            # mask the diagonal block (first 128 cols of tile)
                            nc.vector.tensor_mul(
                                t_h[:, off : off + P], t_h[:, off : off + P], tri_sb
                            )

                        # pass 2: O accumulation + normalize + transpose
                        for qb in range(NKC):
                            po = pso.tile([P, P + 1], F32, tag="pso")
                            for kc in range(qb + 1):
                                col = T_OFFS[kc] + (qb - kc) * P
                                nc.tensor.matmul(
                                    po,
                                    t_h[:, col : col + P],
                                    v_sb[:, kc, h, :],
                                    start=(kc == 0),
                                    stop=(kc == qb),
                                )
                            recip = pbo.tile([P, 1], F32, tag="recip")
                            nc.vector.reciprocal(recip, po[:, P : P + 1])
                            o_sb = pbo.tile([P, P], BF16, tag="o")
                            nc.vector.tensor_scalar_mul(o_sb, po[:, 0:P], recip)
                            i4 = qb % 4
                            if i4 == 0:
                                pt = pst.tile([P, 512], BF16, tag="pst")
                            nc.tensor.transpose(
                                pt[:, i4 * P : (i4 + 1) * P], o_sb, ident
                            )
                            if i4 == 3:
                                g = qb // 4
                                nc.vector.tensor_copy(
                                    ot_sb[:, h, g * 512 : (g + 1) * 512], pt
                                )

                # ---------------- Phase C: output projection ----------------
                for _rc in range(rep_c):
                  with ExitStack() as cctx:
                    stg = cctx.enter_context(tc.tile_pool(name="stg", bufs=3))
                    pco = cctx.enter_context(
                        tc.tile_pool(name="pco", bufs=2, space="PSUM")
                    )
                    for sm in range(NKC):
                        po = pco.tile([P, DM], F32, tag="pco")
                        for f in range(HEADS_PER_CORE):
                            lhsT = ot_sb[:, f, sm * P : (sm + 1) * P]
                            for nd in range(4):
                                nc.tensor.matmul(
                                    po[:, nd * 512 : (nd + 1) * 512],
                                    lhsT,
                                    wo_sb[:, f, nd * 512 : (nd + 1) * 512],
                                    start=(f == 0),
                                    stop=(f == HEADS_PER_CORE - 1),
                                )
                        stage = stg.tile([P, DM], F32, tag="stage")
                        if sm % 2 == 0:
                            nc.vector.tensor_copy(stage, po)
                        else:
                            nc.scalar.copy(stage, po)
                        nc.sync.dma_start(
                            out=out_ap[sm * P : (sm + 1) * P, :], in_=stage
                        )

    nc.compile()
    return nc


def prep_in_maps(x, mask, w_q, w_k, w_v, w_o):
    """Host-side sharding: per-core input dicts (8 cores)."""
    x = np.asarray(x, dtype=np.float32)
    mask = np.asarray(mask, dtype=np.float32)
    w_q = np.asarray(w_q, dtype=np.float32)
    w_k = np.asarray(w_k, dtype=np.float32)
    w_v = np.asarray(w_v, dtype=np.float32)
    w_o = np.asarray(w_o, dtype=np.float32)

    # tri[k, q] = 1 where allowed (k <= q), from the mask's diagonal block
    tri = np.ascontiguousarray(
        (mask[:P, :P].T == 0.0).astype(NPBF16)
    )
    xts = [np.ascontiguousarray(x[b].T).astype(NPBF16) for b in range(2)]
    in_maps = []
    for c in range(8):
        b, j = divmod(c, 4)
        sl = slice(j * F, (j + 1) * F)
        in_maps.append(
            {
                "xt": xts[b],
                "wq": np.ascontiguousarray(w_q[:, sl]).astype(NPBF16),
                "wk": np.ascontiguousarray(w_k[:, sl]).astype(NPBF16),
                "wv": np.ascontiguousarray(w_v[:, sl]).astype(NPBF16),
                "wo": np.ascontiguousarray(w_o[sl, :]).astype(NPBF16),
                "tri": tri,
            }
        )
    return in_maps


def gather(results):
    """Sum the 4 partial outputs per batch element."""
    out = np.zeros((2, SEQ, DM), np.float32)
    for c in range(8):
        out[c // 4] += results[c]["out"]
    return out


_cache = threading.local()


def kernel(x, mask, w_q, w_k, w_v, w_o):
    from concourse.bass_utils import run_bass_kernel_spmd

    nc = getattr(_cache, "nc", None)
    if nc is None:
        nc = build_nc(1)
        _cache.nc = nc
    in_maps = prep_in_maps(x, mask, w_q, w_k, w_v, w_o)
    res = run_bass_kernel_spmd(nc, in_maps, core_ids=list(range(8)))
    return gather(res.results)


# revision 8
# speedup vs baseline: 1.2318x; 1.2318x over previous
"""Multi-head attention (batch=2, seq=2048, d_model=2048, 16 heads, causal)
on 8 Trainium2 NeuronCores.

Sharding (Megatron-style tensor parallel + data parallel):
  core c -> batch b = c // 4, feature block j = c % 4 (4 heads = 512 features).
  Each core computes Q/K/V projections for its 512 feature columns
  (w_q/w_k/w_v column-sliced), attention for its 4 heads, and a partial
  output projection (w_o row-sliced).  The 4 partial outputs per batch
  element are summed on the host (the Megatron row-parallel AllReduce).

Device math (per core), all matmuls in bf16 with fp32 PSUM accumulation:
  xT  = x[b].T                          [2048 dm, 2048 s]   (host-prepped)
  Q^T = wq_c.T @ ... -> lhsT=wq chunks  [512 f, 2048 s]
  K^T                                    [512 f, 2048 s]
  V   = x @ wv_c                         [2048 s, 512 f]    (+ ones column)
  per head h, per key block kc:  S^T[k, q] = K^T_h[:,kc].T @ Q^T_h
  T = exp(S^T / sqrt(128))  (unnormalized softmax; scores are O(5) so no
      max-subtraction is needed in fp32), causal-masked
  per query block qb: O[q, d|sum] = sum_kc T_kc[:, qb].T @ [V_kc | 1]
  O /= sum  -> transpose via PE -> O^T [512 f, 2048 s]
  out partial = O^T.T @ wo_c             [2048 s, 2048 dmo]  fp32
"""

import math
import threading
from contextlib import ExitStack

import ml_dtypes
import numpy as np

import concourse.bass as bass
import concourse.mybir as mybir
import concourse.tile as tile
from concourse import bacc
from concourse.masks import make_identity

import os
_DT = os.environ.get("MHA_DTYPE", "bf16")
BF16 = mybir.dt.float16 if _DT == "fp16" else mybir.dt.bfloat16
F32 = mybir.dt.float32
NPBF16 = np.float16 if _DT == "fp16" else ml_dtypes.bfloat16

SEQ = 2048
DM = 2048
HEADS_PER_CORE = 4
F = 512  # features per core
P = 128
NKC = SEQ // P  # 16 key blocks
NR = DM // P  # 16 contraction chunks
SCALE = 1.0 / math.sqrt(128.0)

# compact T-buffer offsets: block kc covers q in [kc*128, 2048)
T_WIDTHS = [SEQ - kc * P for kc in range(NKC)]
T_OFFS = list(np.cumsum([0] + T_WIDTHS[:-1]))
T_TOTAL = int(np.sum(T_WIDTHS))  # 17408


def build_nc(iters: int = 1, rep_a: int = 1, rep_b: int = 1, rep_c: int = 1) -> bacc.Bacc:
    nc = bacc.Bacc("TRN2", num_devices=8)

    xt_h = nc.dram_tensor("xt", [DM, SEQ], BF16, kind="ExternalInput")
    wq_h = nc.dram_tensor("wq", [DM, F], BF16, kind="ExternalInput")
    wk_h = nc.dram_tensor("wk", [DM, F], BF16, kind="ExternalInput")
    wv_h = nc.dram_tensor("wv", [DM, F], BF16, kind="ExternalInput")
    wo_h = nc.dram_tensor("wo", [F, DM], BF16, kind="ExternalInput")
    tri_h = nc.dram_tensor("tri", [P, P], BF16, kind="ExternalInput")
    out_h = nc.dram_tensor("out", [SEQ, DM], F32, kind="ExternalOutput")

    xt = xt_h.ap()
    wo_r = wo_h.ap().rearrange("(c p) n -> p c n", p=P)  # [128, 4, 2048]
    out_ap = out_h.ap()

    with tile.TileContext(nc) as tc, ExitStack() as octx:
        consts = octx.enter_context(tc.tile_pool(name="consts", bufs=1))
        ident = consts.tile([P, P], BF16)
        make_identity(nc, ident)
        tri_sb = consts.tile([P, P], BF16)
        nc.sync.dma_start(out=tri_sb, in_=tri_h.ap())

        for _ in range(iters):
            with ExitStack() as ictx:
                persist = ictx.enter_context(tc.tile_pool(name="persist", bufs=1))
                qt_sb = persist.tile([P, HEADS_PER_CORE, SEQ], BF16)
                kt_sb = persist.tile([P, HEADS_PER_CORE, SEQ], BF16)
                v_sb = persist.tile([P, NKC, HEADS_PER_CORE, P + 1], BF16)
                ot_sb = persist.tile([P, HEADS_PER_CORE, SEQ], BF16)
                wo_sb = persist.tile([P, HEADS_PER_CORE, DM], BF16)

                nc.sync.dma_start(out=wo_sb, in_=wo_r)
                # ones column for the fused softmax-denominator trick
                nc.vector.memset(v_sb[:, :, :, P : P + 1], 1.0)

                # ---------------- Phase A: projections ----------------
                for _ra in range(rep_a):
                  with ExitStack() as actx:
                    pa = actx.enter_context(tc.tile_pool(name="pa", bufs=1))
                    wpool = actx.enter_context(tc.tile_pool(name="wpool", bufs=2))
                    psq = actx.enter_context(
                        tc.tile_pool(name="psq", bufs=3, space="PSUM")
                    )
                    psv = actx.enter_context(
                        tc.tile_pool(name="psv", bufs=2, space="PSUM")
                    )

                    xt_sb = pa.tile([P, NR, SEQ], BF16)
                    for r in range(NR):
                        nc.sync.dma_start(
                            out=xt_sb[:, r, :], in_=xt[r * P : (r + 1) * P, :]
                        )

                    # Q^T and K^T: psum[f_rel, s] = sum_r w[r,f].T @ xT[r, s]
                    for w_h, dst in ((wq_h, qt_sb), (wk_h, kt_sb)):
                        w_r = w_h.ap().rearrange("(r p) f -> p r f", p=P)
                        w_t = wpool.tile([P, NR, F], BF16, tag="w")
                        for rr in range(4):
                            nc.sync.dma_start(
                                out=w_t[:, rr * 4 : (rr + 1) * 4, :],
                                in_=w_r[:, rr * 4 : (rr + 1) * 4, :],
                            )
                        for f in range(HEADS_PER_CORE):
                            pq0 = psq.tile([P, 1024], F32, tag="psq")
                            pq1 = psq.tile([P, 1024], F32, tag="psq")
                            for r in range(NR):
                                lhsT = w_t[:, r, f * P : (f + 1) * P]
                                for half, pq in ((0, pq0), (1, pq1)):
                                    for sn in range(2):
                                        s0 = half * 1024 + sn * 512
                                        nc.tensor.matmul(
                                            pq[:, sn * 512 : (sn + 1) * 512],
                                            lhsT,
                                            xt_sb[:, r, s0 : s0 + 512],
                                            start=(r == 0),
                                            stop=(r == NR - 1),
                                        )
                            nc.vector.tensor_copy(dst[:, f, 0:1024], pq0)
                            nc.scalar.copy(dst[:, f, 1024:2048], pq1)

                    # V: psum[s_rel, f] = sum_r xT[r, s].T @ wv[r, f]
                    wv_r = wv_h.ap().rearrange("(r p) f -> p r f", p=P)
                    wv_t = wpool.tile([P, NR, F], BF16, tag="w")
                    for rr in range(4):
                        nc.sync.dma_start(
                            out=wv_t[:, rr * 4 : (rr + 1) * 4, :],
                            in_=wv_r[:, rr * 4 : (rr + 1) * 4, :],
                        )
                    for sm in range(NKC):
                        pv = psv.tile([P, F], F32, tag="psv")
                        for r in range(NR):
                            nc.tensor.matmul(
                                pv,
                                xt_sb[:, r, sm * P : (sm + 1) * P],
                                wv_t[:, r, :],
                                start=(r == 0),
                                stop=(r == NR - 1),
                            )
                        eng = nc.vector if sm % 2 == 0 else nc.scalar
                        if eng is nc.vector:
                            eng.tensor_copy(
                                v_sb[:, sm, :, 0:P],
                                pv.rearrange("p (h d) -> p h d", h=HEADS_PER_CORE),
                            )
                        else:
                            eng.copy(
                                v_sb[:, sm, :, 0:P],
                                pv.rearrange("p (h d) -> p h d", h=HEADS_PER_CORE),
                            )

                # ---------------- Phase B: attention ----------------
                for _rb in range(rep_b):
                  with ExitStack() as bctx:
                    pb = bctx.enter_context(tc.tile_pool(name="pb", bufs=2))
                    pbo = bctx.enter_context(tc.tile_pool(name="pbo", bufs=3))
                    pss = bctx.enter_context(
                        tc.tile_pool(name="pss", bufs=2, space="PSUM")
                    )
                    pso = bctx.enter_context(
                        tc.tile_pool(name="pso", bufs=2, space="PSUM")
                    )
                    pst = bctx.enter_context(
                        tc.tile_pool(name="pst", bufs=2, space="PSUM")
                    )

                    for h in range(HEADS_PER_CORE):
                        t_h = pb.tile([P, T_TOTAL], BF16, tag="T")
                        # pass 1: T = causal_mask(exp(S^T / sqrt(d)))
                        for kc in range(NKC):
                            w = T_WIDTHS[kc]
                            off = T_OFFS[kc]
                            q0 = kc * P
                            lhsT = kt_sb[:, h, kc * P : (kc + 1) * P]
                            for c in range((w + 1023) // 1024):
                                wc = min(1024, w - c * 1024)
                                ps = pss.tile([P, 1024], F32, tag="pss")
                                for n in range((wc + 511) // 512):
                                    nw = min(512, wc - n * 512)
                                    nc.tensor.matmul(
                                        ps[:, n * 512 : n * 512 + nw],
                                        lhsT,
                                        qt_sb[
                                            :,
                                            h,
                                            q0 + c * 1024 + n * 512 : q0
                                            + c * 1024
                                            + n * 512
                                            + nw,
                                        ],
                                        start=True,
                                        stop=True,
                                    )
                                nc.scalar.activation(
                                    t_h[:, off + c * 1024 : off + c * 1024 + wc],
                                    ps[:, 0:wc],
                                    mybir.ActivationFunctionType.Exp,
                                    scale=SCALE,
                                )
                            # mask the diagonal block (first 128 cols of tile)
                            nc.vector.tensor_mul(
                                t_h[:, off : off + P], t_h[:, off : off + P], tri_sb
                            )

                        # pass 2: O accumulation + normalize + transpose
                        for qb in range(NKC):
                            po = pso.tile([P, P + 1], F32, tag="pso")
                            for kc in range(qb + 1):
                                col = T_OFFS[kc] + (qb - kc) * P
                                nc.tensor.matmul(
                                    po,
                                    t_h[:, col : col + P],
                                    v_sb[:, kc, h, :],
                                    start=(kc == 0),
                                    stop=(kc == qb),
                                )
                            recip = pbo.tile([P, 1], F32, tag="recip")
                            nc.vector.reciprocal(recip, po[:, P : P + 1])
                            o_sb = pbo.tile([P, P], BF16, tag="o")
                            nc.vector.tensor_scalar_mul(o_sb, po[:, 0:P], recip)
                            i4 = qb % 4
                            if i4 == 0:
                                pt = pst.tile([P, 512], BF16, tag="pst")
                            nc.tensor.transpose(
                                pt[:, i4 * P : (i4 + 1) * P], o_sb, ident
                            )
                            if i4 == 3:
                                g = qb // 4
                                nc.vector.tensor_copy(
                                    ot_sb[:, h, g * 512 : (g + 1) * 512], pt
                                )

                # ---------------- Phase C: output projection ----------------
                for _rc in range(rep_c):
                  with ExitStack() as cctx:
                    stg = cctx.enter_context(tc.tile_pool(name="stg", bufs=3))
                    pco = cctx.enter_context(
                        tc.tile_pool(name="pco", bufs=2, space="PSUM")
                    )
                    for sm in range(NKC):
                        po = pco.tile([P, DM], F32, tag="pco")
                        for f in range(HEADS_PER_CORE):
                            lhsT = ot_sb[:, f, sm * P : (sm + 1) * P]
                            for nd in range(4):
                                nc.tensor.matmul(
                                    po[:, nd * 512 : (nd + 1) * 512],
                                    lhsT,
                                    wo_sb[:, f, nd * 512 : (nd + 1) * 512],
                                    start=(f == 0),
                                    stop=(f == HEADS_PER_CORE - 1),
                                )
                        stage = stg.tile([P, DM], F32, tag="stage")
                        if sm % 2 == 0:
                            nc.vector.tensor_copy(stage, po)
                        else:
                            nc.scalar.copy(stage, po)
                        nc.sync.dma_start(
                            out=out_ap[sm * P : (sm + 1) * P, :], in_=stage
                        )

    nc.compile()
    return nc


def prep_in_maps(x, mask, w_q, w_k, w_v, w_o):
    """Host-side sharding: per-core input dicts (8 cores)."""
    x = np.asarray(x, dtype=np.float32)
    mask = np.asarray(mask, dtype=np.float32)
    w_q = np.asarray(w_q, dtype=np.float32)
    w_k = np.asarray(w_k, dtype=np.float32)
    w_v = np.asarray(w_v, dtype=np.float32)
    w_o = np.asarray(w_o, dtype=np.float32)

    # tri[k, q] = 1 where allowed (k <= q), from the mask's diagonal block
    tri = np.ascontiguousarray(
        (mask[:P, :P].T == 0.0).astype(NPBF16)
    )
    xts = [np.ascontiguousarray(x[b].T).astype(NPBF16) for b in range(2)]
    in_maps = []
    for c in range(8):
        b, j = divmod(c, 4)
        sl = slice(j * F, (j + 1) * F)
        in_maps.append(
            {
                "xt": xts[b],
                "wq": np.ascontiguousarray(w_q[:, sl]).astype(NPBF16),
                "wk": np.ascontiguousarray(w_k[:, sl]).astype(NPBF16),
                "wv": np.ascontiguousarray(w_v[:, sl]).astype(NPBF16),
                "wo": np.ascontiguousarray(w_o[sl, :]).astype(NPBF16),
                "tri": tri,
            }
        )
    return in_maps


def gather(results):
    """Sum the 4 partial outputs per batch element."""
    out = np.zeros((2, SEQ, DM), np.float32)
    for c in range(8):
        out[c // 4] += results[c]["out"]
    return out


_cache = threading.local()


def kernel(x, mask, w_q, w_k, w_v, w_o):
    from concourse.bass_utils import run_bass_kernel_spmd

    nc = getattr(_cache, "nc", None)
    if nc is None:
        nc = build_nc(1)
        _cache.nc = nc
    in_maps = prep_in_maps(x, mask, w_q, w_k, w_v, w_o)
    res = run_bass_kernel_spmd(nc, in_maps, core_ids=list(range(8)))
    return gather(res.results)
